# revision 26
# baseline (speedup 1.0000x reference)
"""Causal self-attention on 8 Trainium2 NeuronCores.

Problem (hardcoded): B=4, T=2048, C=1024, H=16, D=64.
  qkv = x @ w_qkv + b_qkv ; per-head causal softmax attention ; out = attn @ w_proj + b_proj

Sharding (per hint): tensor-parallel over heads x data-parallel over batch.
  core c -> batch b = c // 2, head group g = c % 2 (heads g*8 .. g*8+7).
Each core computes QKV for its 8 heads, causal attention, and a partial
projection (its 512 input channels of w_proj). Host sums the two partials per
batch and adds b_proj.

On-core layout ("transposed" attention so softmax reduction lands on the
matmul contraction axis):
  xT   [C, T]  (host pre-transposed, bf16)
  QT,KT [d, t] per head, 2 heads stacked per 128 partitions
  V_aug [t, 65] per head (col 64 = ones -> PV matmul emits softmax denom)
  S^T  [j, i] tiles from lhsT=KT, rhs=QT (K=64 contraction); the head pair's
       two S tiles land in one [128, 2, 512] PSUM tile (2 banks) so a single
       Exp activation serves both heads; the two S matmuls row-tile (rows
       0-63 / 64-127) and run concurrently on the PE.
  P = exp(S^T/8) (ScalarE, PSUM->SBUF bf16); diagonal-band tiles are trimmed
       to their live i-range (columns i < 128*(jt-4ci) are fully causal-masked
       -> skipped in S, exp, and PV); the diagonal crossing itself is masked
       by one persistent [128, 2, 128] 0/1 triangle multiply (VectorE).
  O_aug^T [65, i] accumulated over j chunks per head; row 64 = denominator.
  AT = O^T * (1/denom) broadcast -> proj lhsT; partial = A @ w_proj_slice.

Scheduling: DMA is emitted in consumption order (wv, xT t-quarters, per-pair
wq/wk, wp) so the first V matmuls start ~7us in; V/QK/proj fill work is
interleaved into the exp-paced attention stretches; proj for chunk ci is
deferred until after the next chunk's first pairs so the softmax-normalize
chain never blocks the PE queue head.
"""

import numpy as np
import ml_dtypes

B, T, C, H, D = 4, 2048, 1024, 16, 64
HL = H // 2          # heads per core
CL = HL * D          # local channels (512)
NPAIR = HL // 2      # head pairs per core (4)
CCH = C // 128       # contraction chunks for qkv (8)
PCH = CL // 128      # contraction chunks for proj (4)
TT = T // 128        # t tiles (16)
NI = T // 512        # i chunks (4)
N_CORES = 8
BF16 = ml_dtypes.bfloat16

_compiled = None


def _build(nc):
    import concourse.tile as tile
    from concourse import mybir

    bf = mybir.dt.bfloat16
    f32 = mybir.dt.float32
    Exp = mybir.ActivationFunctionType.Exp

    # xT host-packed quarter-major: [128 part, 4 q, CCH, 512] (8KB DMA lines)
    xT = nc.dram_tensor("xT", [128, 4 * CCH * 512], bf, kind="ExternalInput").ap()
    # wq/wk host-packed per pair: [NPAIR, 128 part, CCH*128] (2KB DMA lines)
    wq = nc.dram_tensor("wq", [NPAIR, 128, CCH * 128], bf, kind="ExternalInput").ap()
    wk = nc.dram_tensor("wk", [NPAIR, 128, CCH * 128], bf, kind="ExternalInput").ap()
    # wv host-packed: [128 part, CCH*CL] (8KB DMA lines)
    wv = nc.dram_tensor("wv", [128, CCH * CL], bf, kind="ExternalInput").ap()
    bq = nc.dram_tensor("bq", [128, NPAIR], f32, kind="ExternalInput").ap()
    bk = nc.dram_tensor("bk", [128, NPAIR], f32, kind="ExternalInput").ap()
    bv = nc.dram_tensor("bv", [128, CL], f32, kind="ExternalInput").ap()
    wp = nc.dram_tensor("wp", [CL, C], bf, kind="ExternalInput").ap()
    out = nc.dram_tensor("out", [T, C], bf, kind="ExternalOutput").ap()

    xT_r = xT.rearrange("p (q cc t) -> p q cc t", q=4, cc=CCH)
    wv_r = wv.rearrange("p (cc m) -> p cc m", cc=CCH)
    wp_r = wp.rearrange("(cc p) n -> p cc n", p=128)
    wq_r = wq.rearrange("a p (cc m) -> a p cc m", m=128)
    wk_r = wk.rearrange("a p (cc m) -> a p cc m", m=128)

    with tile.TileContext(nc) as tc:
        import contextlib

        with contextlib.ExitStack() as ctx:
            persist = ctx.enter_context(tc.tile_pool(name="persist", bufs=1))
            # PSUM: tag "s" slots are [128, 2, 512] (2 banks) x 3 bufs = 6
            # banks; o0/o1 are 1 bank x 1 buf each -> 8 banks total.  The o
            # tiles are staged to SBUF by one early copy so single-buffering
            # them costs little, and the 3-deep s ring gives the PE lookahead
            # past the exp drain rate.
            ps_pool = ctx.enter_context(tc.tile_pool(name="ps_pool", bufs=3, space="PSUM"))
            o_ps = ctx.enter_context(tc.tile_pool(name="o_ps", bufs=1, space="PSUM"))
            p_pool = ctx.enter_context(tc.tile_pool(name="p_pool", bufs=3))
            r_pool = ctx.enter_context(tc.tile_pool(name="r_pool", bufs=2))
            st_pool = ctx.enter_context(tc.tile_pool(name="st_pool", bufs=3))

            # ---- persistent SBUF tensors ----
            xT_sb = persist.tile([128, 4, CCH, 512], bf)
            wq_sb = persist.tile([128, NPAIR, CCH, 128], bf)
            wk_sb = persist.tile([128, NPAIR, CCH, 128], bf)
            wv_sb = persist.tile([128, CCH, CL], bf)
            wp_sb = persist.tile([128, PCH, C], bf)
            bq_sb = persist.tile([128, NPAIR], f32)
            bk_sb = persist.tile([128, NPAIR], f32)
            bv_sb = persist.tile([128, CL], f32)
            QT_sb = persist.tile([128, NPAIR, T], bf)
            KT_sb = persist.tile([128, NPAIR, T], bf)
            V_sb = persist.tile([128, TT, HL, D + 1], bf)
            AT_sb = persist.tile([128, PCH, T], bf)
            tri_sb = persist.tile([128, 2, 128], bf)

            # ---- DMA emission in consumption order ----
            # 1. wv + bv (first consumers: V matmuls); halves so the first
            # V accumulation chain can start on cc 0-3 early
            nc.sync.dma_start(out=bv_sb[:], in_=bv[:])
            nc.sync.dma_start(out=wv_sb[:, 0:4], in_=wv_r[:, 0:4])
            nc.sync.dma_start(out=xT_sb[:, 0, 0:4], in_=xT_r[:, 0, 0:4])
            nc.sync.dma_start(out=wv_sb[:, 4:8], in_=wv_r[:, 4:8])
            nc.sync.dma_start(out=xT_sb[:, 0, 4:8], in_=xT_r[:, 0, 4:8])
            # 2. pair-0 qk weights
            nc.sync.dma_start(out=wq_sb[:, 0], in_=wq_r[0])
            nc.sync.dma_start(out=wk_sb[:, 0], in_=wk_r[0])
            # 3. xT quarter 1 + biases
            nc.sync.dma_start(out=xT_sb[:, 1], in_=xT_r[:, 1])
            nc.sync.dma_start(out=bq_sb[:], in_=bq[:])
            nc.sync.dma_start(out=bk_sb[:], in_=bk[:])
            # 5. remaining qk weights
            for p in range(1, NPAIR):
                nc.sync.dma_start(out=wq_sb[:, p], in_=wq_r[p])
                nc.sync.dma_start(out=wk_sb[:, p], in_=wk_r[p])
            # 6. xT quarters 2, 3
            nc.sync.dma_start(out=xT_sb[:, 2], in_=xT_r[:, 2])
            nc.sync.dma_start(out=xT_sb[:, 3], in_=xT_r[:, 3])
            # 7. proj weights
            for cc in range(PCH):
                nc.sync.dma_start(out=wp_sb[:, cc, :], in_=wp_r[:, cc, :])

            # causal 0/1 triangle, replicated for the pair dim:
            # tri[jj, :, ii] = 1 if ii >= jj else 0 (same for every band tile)
            nc.vector.memset(tri_sb[:], 1.0)
            nc.gpsimd.affine_select(
                out=tri_sb[:],
                in_=tri_sb[:],
                compare_op=mybir.AluOpType.is_ge,
                fill=0.0,
                base=0,
                pattern=[[0, 2], [1, 128]],
                channel_multiplier=-1,
            )
            # ones column of V_aug
            nc.vector.memset(V_sb[:, :, :, D], 1.0)

            # ---- fill units: self-contained PE work parcels that can be
            # interleaved into the exp-paced attention stretches ----

            # V projection for one t-tile (8 MMs + bias add, ~1.7us PE)
            def v_unit(tt):
                def f():
                    ps = ps_pool.tile([128, 512], f32, tag="s")
                    q, off = tt // 4, (tt % 4) * 128
                    for cc in range(CCH):
                        nc.tensor.matmul(
                            ps[:],
                            lhsT=xT_sb[:, q, cc, off : off + 128],
                            rhs=wv_sb[:, cc, :],
                            start=(cc == 0),
                            stop=(cc == CCH - 1),
                        )
                    nc.vector.tensor_add(
                        V_sb[:, tt, :, 0:D],
                        ps[:].rearrange("p (h d) -> p h d", h=HL),
                        bv_sb[:].rearrange("p (h d) -> p h d", h=HL),
                    )
                return f

            # Q or K projection for one head pair / one 1024-wide t half
            # (16 MMs + bias add, ~3.4us PE)
            def qk_unit(pair, th, which):
                def f():
                    w_sb, dst, b_sb = (
                        (wq_sb, QT_sb, bq_sb),
                        (wk_sb, KT_sb, bk_sb),
                    )[which]
                    ps = ps_pool.tile([128, 2, 512], f32, tag="s")
                    for h2 in range(2):
                        q = th * 2 + h2
                        for cc in range(CCH):
                            nc.tensor.matmul(
                                ps[:, h2, :],
                                lhsT=w_sb[:, pair, cc, :],
                                rhs=xT_sb[:, q, cc, :],
                                start=(cc == 0),
                                stop=(cc == CCH - 1),
                            )
                    nc.vector.tensor_scalar_add(
                        dst[:, pair, th * 1024 : (th + 1) * 1024],
                        ps[:].rearrange("p a b -> p (a b)"),
                        b_sb[:, pair : pair + 1],
                    )
                return f

            # half a projection t-tile (4 MMs + copy, DMA on the second
            # half, ~0.9us PE)
            def proj_unit(tt, nh, so_box):
                def f():
                    if nh == 0:
                        so = st_pool.tile([128, 1024], bf, tag="so", name="so")
                        so_box["t"] = so
                    so = so_box["t"]
                    ps = ps_pool.tile([128, 512], f32, tag="s")
                    for cc in range(PCH):
                        nc.tensor.matmul(
                            ps[:],
                            lhsT=AT_sb[:, cc, tt * 128 : (tt + 1) * 128],
                            rhs=wp_sb[:, cc, nh * 512 : (nh + 1) * 512],
                            start=(cc == 0),
                            stop=(cc == PCH - 1),
                        )
                    nc.vector.tensor_copy(so[:, nh * 512 : (nh + 1) * 512], ps[:])
                    if nh == 1:
                        nc.sync.dma_start(
                            out=out[tt * 128 : (tt + 1) * 128, :], in_=so[:]
                        )
                return f

            # deadline-ordered fill queue; (ci, pair) lexicographic deadlines
            fills = []
            fseq = [0]

            def push_fill(deadline, fn):
                fills.append((deadline, fseq[0], fn))
                fseq[0] += 1
                fills.sort(key=lambda x: (x[0], x[1]))

            def proj_units(ci, deadline):
                for tt in range(4 * ci, 4 * ci + 4):
                    box = {}
                    for nh in range(2):
                        push_fill(deadline, proj_unit(tt, nh, box))

            def pop_fill():
                if fills:
                    fills.pop(0)[2]()

            def drain_fills(ci, pair):
                while fills and fills[0][0] <= (ci, pair):
                    fills.pop(0)[2]()

            # attention for one head pair / one 512-wide i chunk, with the
            # diagonal band trimmed to its live i-range
            def emit_att(pair, ci):
                o0 = o_ps.tile([D + 1, 512], f32, tag="o0")
                o1 = o_ps.tile([D + 1, 512], f32, tag="o1")
                njt = 4 * (ci + 1)

                # PV for tile jt (P read from SBUF pt, trimmed to [i0:512))
                def emit_pv(jt, i0, pt):
                    for s, ot in enumerate((o0, o1)):
                        nc.tensor.matmul(
                            ot[:, i0:512],
                            lhsT=V_sb[:, jt, 2 * pair + s, :],
                            rhs=pt[:, s, i0:512],
                            start=(jt == 0),
                            stop=(jt == njt - 1),
                            skip_group_check=True,
                        )

                # software-pipelined: the (always-ready) S pair for jt+1 is
                # emitted before PV for jt, so PV never blocks the PE queue
                # head while exp(jt) is still draining
                prev = None
                for jt in range(njt):
                    r = jt - 4 * ci
                    i0 = max(0, 128 * r)  # cols i < 128r are fully masked
                    st = ps_pool.tile([128, 2, 512], f32, tag="s")
                    for s in range(2):
                        nc.tensor.matmul(
                            st[:, s, i0:512],
                            lhsT=KT_sb[
                                64 * s : 64 * (s + 1),
                                pair,
                                jt * 128 : (jt + 1) * 128,
                            ],
                            rhs=QT_sb[
                                64 * s : 64 * (s + 1),
                                pair,
                                ci * 512 + i0 : (ci + 1) * 512,
                            ],
                            start=True,
                            stop=True,
                        )
                    pt = p_pool.tile([128, 2, 512], bf, tag="p")
                    nc.scalar.activation(
                        pt[:, :, i0:512], st[:, :, i0:512], Exp, scale=0.125
                    )
                    if r >= 0:
                        nc.vector.tensor_mul(
                            pt[:, :, i0 : i0 + 128],
                            pt[:, :, i0 : i0 + 128],
                            tri_sb[:],
                        )
                    if prev is not None:
                        emit_pv(*prev)
                    prev = (jt, i0, pt)
                    if jt % 5 == 2 and jt < 4 * ci:
                        pop_fill()
                emit_pv(*prev)
                # early PSUM->SBUF copies free the (single-buffered) o slots;
                # the PSUM reads legally shift head1's rows to partitions
                # 64-127 so the rest of the chain is partition-aligned
                oco = st_pool.tile([128, 512], f32, tag="oc")
                for s, ot in enumerate((o0, o1)):
                    nc.vector.tensor_copy(oco[64 * s : 64 * (s + 1), :], ot[0:D, :])
                    dn = r_pool.tile([1, 512], f32, tag=f"dn{s}")
                    nc.vector.tensor_copy(dn[:], ot[D : D + 1, :])
                    rc = r_pool.tile([1, 512], f32, tag=f"rc{s}")
                    nc.vector.reciprocal_approx_fast(rc[:], dn[:])
                    rb = r_pool.tile([128, 512], f32, tag=f"rb{s}")
                    nc.gpsimd.partition_broadcast(rb[:], rc[:])
                    nc.vector.tensor_mul(
                        AT_sb[
                            64 * s : 64 * (s + 1),
                            pair,
                            ci * 512 : (ci + 1) * 512,
                        ],
                        oco[64 * s : 64 * (s + 1), :],
                        rb[64 * s : 64 * (s + 1), :],
                    )

            # ---- main schedule ----
            # Structural (pre-attention) work: V tiles 0-3 and pair-0 th0 QK.
            # Everything else enters the fill queue and is popped one unit
            # per ~5 attention tiles, with deadline drains before the
            # attention that needs it.
            for tt in range(4):
                v_unit(tt)()
            qk_unit(0, 0, 0)()
            qk_unit(0, 0, 1)()
            for p in range(1, NPAIR):
                for w in (0, 1):
                    push_fill((0, p), qk_unit(p, 0, w))
            for ci in range(NI):
                if ci <= 2:
                    # V tiles for the next chunk, wanted by its first pair
                    for tt in range(4 * ci + 4, 4 * ci + 8):
                        push_fill((ci + 1, 0), v_unit(tt))
                if ci == 1:
                    # second-half QK, wanted per pair at chunk 2
                    for p in range(NPAIR):
                        for w in (0, 1):
                            push_fill((2, p), qk_unit(p, 1, w))
                for pair in range(NPAIR):
                    drain_fills(ci, pair)
                    emit_att(pair, ci)
                    if ci >= 1 and pair == 0:
                        # previous chunk's projection (AT fully written)
                        proj_units(ci - 1, (ci, 99))
            drain_fills(99, 99)
            proj_units(NI - 1, (99, 100))
            drain_fills(99, 100)
    return nc


def _get_compiled():
    global _compiled
    if _compiled is None:
        from concourse import bacc

        nc = bacc.Bacc(
            "TRN2", target_bir_lowering=False, debug=False, num_devices=N_CORES
        )
        _build(nc)
        nc.compile()
        _compiled = nc
    return _compiled


def _shard_inputs(x, w_qkv, b_qkv, w_proj):
    """Build the 8 per-core input dicts (host-side transpose/slice/cast)."""
    in_maps = []
    wq_f, wk_f, wv_f = w_qkv[:, :C], w_qkv[:, C : 2 * C], w_qkv[:, 2 * C :]
    for c in range(N_CORES):
        b, g = c // 2, c % 2
        sl = slice(g * CL, (g + 1) * CL)
        bqs = np.ascontiguousarray(b_qkv[0 * C :][sl].reshape(NPAIR, 128).T)
        bks = np.ascontiguousarray(b_qkv[1 * C :][sl].reshape(NPAIR, 128).T)
        bvs = np.ascontiguousarray(
            np.broadcast_to(b_qkv[2 * C :][sl][None, :], (128, CL))
        )
        # per-pair packed qk weights: [NPAIR, 128 part, CCH*128], where the
        # partition index runs over the 128 rows of each 128-chunk of C
        wq_p = np.ascontiguousarray(
            wq_f[:, sl].reshape(CCH, 128, NPAIR, 128).transpose(2, 1, 0, 3)
            .reshape(NPAIR, 128, CCH * 128)
        )
        wk_p = np.ascontiguousarray(
            wk_f[:, sl].reshape(CCH, 128, NPAIR, 128).transpose(2, 1, 0, 3)
            .reshape(NPAIR, 128, CCH * 128)
        )
        # xT quarter-major: [128 part, 4 q, CCH, 512]
        xT_p = np.ascontiguousarray(
            x[b].T.reshape(CCH, 128, 4, 512).transpose(1, 2, 0, 3)
            .reshape(128, 4 * CCH * 512)
        )
        # wv: [128 part, CCH, CL]
        wv_p = np.ascontiguousarray(
            wv_f[:, sl].reshape(CCH, 128, CL).transpose(1, 0, 2)
            .reshape(128, CCH * CL)
        )
        in_maps.append(
            {
                "xT": xT_p.astype(BF16),
                "wq": wq_p.astype(BF16),
                "wk": wk_p.astype(BF16),
                "wv": wv_p.astype(BF16),
                "bq": bqs.astype(np.float32),
                "bk": bks.astype(np.float32),
                "bv": bvs.astype(np.float32),
                "wp": np.ascontiguousarray(w_proj[sl, :]).astype(BF16),
            }
        )
    return in_maps


def kernel(x, w_qkv, b_qkv, w_proj, b_proj, _trace=False, _tmpdir=None):
    from concourse.bass_utils import run_bass_kernel_spmd

    x = np.asarray(x, dtype=np.float32)
    w_qkv = np.asarray(w_qkv, dtype=np.float32)
    b_qkv = np.asarray(b_qkv, dtype=np.float32)
    w_proj = np.asarray(w_proj, dtype=np.float32)
    b_proj = np.asarray(b_proj, dtype=np.float32)

    nc = _get_compiled()
    in_maps = _shard_inputs(x, w_qkv, b_qkv, w_proj)
    res = run_bass_kernel_spmd(
        nc,
        in_maps,
        core_ids=list(range(N_CORES)),
        trace=_trace,
        tmpdir=_tmpdir,
    )
    out = np.empty((B, T, C), dtype=np.float32)
    for b in range(B):
        out[b] = (
            res.results[2 * b]["out"].astype(np.float32)
            + res.results[2 * b + 1]["out"].astype(np.float32)
            + b_proj
        )
    kernel._last_result = res
    return out


# revision 28
# speedup vs baseline: 1.1188x; 1.1188x over previous
"""Causal self-attention on 8 Trainium2 NeuronCores.

Problem (hardcoded): B=4, T=2048, C=1024, H=16, D=64.
  qkv = x @ w_qkv + b_qkv ; per-head causal softmax attention ; out = attn @ w_proj + b_proj

Sharding (per hint): tensor-parallel over heads x data-parallel over batch.
  core c -> batch b = c // 2, head group g = c % 2 (heads g*8 .. g*8+7).
Each core computes QKV for its 8 heads, causal attention, and a partial
projection (its 512 input channels of w_proj). Host sums the two partials per
batch and adds b_proj.

On-core layout ("transposed" attention so softmax reduction lands on the
matmul contraction axis):
  xT   [C, T]  (host pre-transposed, bf16)
  QT,KT [d, t] per head, 2 heads stacked per 128 partitions
  V_aug [t, 65] per head (col 64 = ones -> PV matmul emits softmax denom)
  S^T  [j, i] tiles from lhsT=KT, rhs=QT (K=64 contraction); the head pair's
       two S tiles land in one [128, 2, 512] PSUM tile (2 banks) so a single
       Exp activation serves both heads; the two S matmuls row-tile (rows
       0-63 / 64-127) and run concurrently on the PE.
  P = exp(S^T/8) (ScalarE, PSUM->SBUF bf16); diagonal-band tiles are trimmed
       to their live i-range (columns i < 128*(jt-4ci) are fully causal-masked
       -> skipped in S, exp, and PV); the diagonal crossing itself is masked
       by one persistent [128, 2, 128] 0/1 triangle multiply (VectorE).
  O_aug^T [65, i] accumulated over j chunks per head; row 64 = denominator.
  AT = O^T * (1/denom) broadcast -> proj lhsT; partial = A @ w_proj_slice.

Scheduling: DMA is emitted in consumption order (wv, xT t-quarters, per-pair
wq/wk, wp) so the first V matmuls start ~7us in; V/QK/proj fill work is
interleaved into the exp-paced attention stretches; proj for chunk ci is
deferred until after the next chunk's first pairs so the softmax-normalize
chain never blocks the PE queue head.
"""

import numpy as np
import ml_dtypes

B, T, C, H, D = 4, 2048, 1024, 16, 64
HL = H // 2          # heads per core
CL = HL * D          # local channels (512)
NPAIR = HL // 2      # head pairs per core (4)
CCH = C // 128       # contraction chunks for qkv (8)
PCH = CL // 128      # contraction chunks for proj (4)
TT = T // 128        # t tiles (16)
NI = T // 512        # i chunks (4)
N_CORES = 8
BF16 = ml_dtypes.bfloat16

_compiled = None


def _build(nc):
    import concourse.tile as tile
    from concourse import mybir

    bf = mybir.dt.bfloat16
    f32 = mybir.dt.float32
    Exp = mybir.ActivationFunctionType.Exp

    # xT host-packed quarter-major: [128 part, 4 q, CCH, 512] (8KB DMA lines)
    xT = nc.dram_tensor("xT", [128, 4 * CCH * 512], bf, kind="ExternalInput").ap()
    # wq/wk host-packed per pair: [NPAIR, 128 part, CCH*128] (2KB DMA lines)
    wq = nc.dram_tensor("wq", [NPAIR, 128, CCH * 128], bf, kind="ExternalInput").ap()
    wk = nc.dram_tensor("wk", [NPAIR, 128, CCH * 128], bf, kind="ExternalInput").ap()
    # wv host-packed: [128 part, CCH*CL] (8KB DMA lines)
    wv = nc.dram_tensor("wv", [128, CCH * CL], bf, kind="ExternalInput").ap()
    bq = nc.dram_tensor("bq", [128, NPAIR], f32, kind="ExternalInput").ap()
    bk = nc.dram_tensor("bk", [128, NPAIR], f32, kind="ExternalInput").ap()
    bv = nc.dram_tensor("bv", [128, CL], f32, kind="ExternalInput").ap()
    wp = nc.dram_tensor("wp", [CL, C], bf, kind="ExternalInput").ap()
    out = nc.dram_tensor("out", [T, C], bf, kind="ExternalOutput").ap()

    xT_r = xT.rearrange("p (q cc t) -> p q cc t", q=4, cc=CCH)
    wv_r = wv.rearrange("p (cc m) -> p cc m", cc=CCH)
    wp_r = wp.rearrange("(cc p) n -> p cc n", p=128)
    wq_r = wq.rearrange("a p (cc m) -> a p cc m", m=128)
    wk_r = wk.rearrange("a p (cc m) -> a p cc m", m=128)

    with tile.TileContext(nc) as tc:
        import contextlib

        with contextlib.ExitStack() as ctx:
            persist = ctx.enter_context(tc.tile_pool(name="persist", bufs=1))
            # PSUM: tag "s" slots are [128, 2, 512] (2 banks) x 3 bufs = 6
            # banks; o0/o1 are 1 bank x 1 buf each -> 8 banks total.  The o
            # tiles are staged to SBUF by one early copy so single-buffering
            # them costs little, and the 3-deep s ring gives the PE lookahead
            # past the exp drain rate.
            ps_pool = ctx.enter_context(tc.tile_pool(name="ps_pool", bufs=3, space="PSUM"))
            o_ps = ctx.enter_context(tc.tile_pool(name="o_ps", bufs=1, space="PSUM"))
            p_pool = ctx.enter_context(tc.tile_pool(name="p_pool", bufs=3))
            r_pool = ctx.enter_context(tc.tile_pool(name="r_pool", bufs=2))
            st_pool = ctx.enter_context(tc.tile_pool(name="st_pool", bufs=3))

            # ---- persistent SBUF tensors ----
            xT_sb = persist.tile([128, 4, CCH, 512], bf)
            wq_sb = persist.tile([128, NPAIR, CCH, 128], bf)
            wk_sb = persist.tile([128, NPAIR, CCH, 128], bf)
            wv_sb = persist.tile([128, CCH, CL], bf)
            wp_sb = persist.tile([128, PCH, C], bf)
            bq_sb = persist.tile([128, NPAIR], f32)
            bk_sb = persist.tile([128, NPAIR], f32)
            bv_sb = persist.tile([128, CL], f32)
            QT_sb = persist.tile([128, NPAIR, T], bf)
            KT_sb = persist.tile([128, NPAIR, T], bf)
            V_sb = persist.tile([128, TT, HL, D + 1], bf)
            AT_sb = persist.tile([128, PCH, T], bf)
            tri_sb = persist.tile([128, 2, 128], bf)

            # ---- DMA emission in consumption order ----
            # 1. wv + bv (first consumers: V matmuls); halves so the first
            # V accumulation chain can start on cc 0-3 early
            nc.sync.dma_start(out=bv_sb[:], in_=bv[:])
            nc.sync.dma_start(out=wv_sb[:, 0:4], in_=wv_r[:, 0:4])
            nc.sync.dma_start(out=xT_sb[:, 0, 0:4], in_=xT_r[:, 0, 0:4])
            nc.sync.dma_start(out=wv_sb[:, 4:8], in_=wv_r[:, 4:8])
            nc.sync.dma_start(out=xT_sb[:, 0, 4:8], in_=xT_r[:, 0, 4:8])
            # 2. pair-0 qk weights
            nc.sync.dma_start(out=wq_sb[:, 0], in_=wq_r[0])
            nc.sync.dma_start(out=wk_sb[:, 0], in_=wk_r[0])
            # 3. xT quarter 1 + biases
            nc.sync.dma_start(out=xT_sb[:, 1], in_=xT_r[:, 1])
            nc.sync.dma_start(out=bq_sb[:], in_=bq[:])
            nc.sync.dma_start(out=bk_sb[:], in_=bk[:])
            # 5. remaining qk weights
            for p in range(1, NPAIR):
                nc.sync.dma_start(out=wq_sb[:, p], in_=wq_r[p])
                nc.sync.dma_start(out=wk_sb[:, p], in_=wk_r[p])
            # 6. xT quarters 2, 3
            nc.sync.dma_start(out=xT_sb[:, 2], in_=xT_r[:, 2])
            nc.sync.dma_start(out=xT_sb[:, 3], in_=xT_r[:, 3])
            # 7. proj weights
            for cc in range(PCH):
                nc.sync.dma_start(out=wp_sb[:, cc, :], in_=wp_r[:, cc, :])

            # causal 0/1 triangle, replicated for the pair dim:
            # tri[jj, :, ii] = 1 if ii >= jj else 0 (same for every band tile)
            nc.vector.memset(tri_sb[:], 1.0)
            nc.gpsimd.affine_select(
                out=tri_sb[:],
                in_=tri_sb[:],
                compare_op=mybir.AluOpType.is_ge,
                fill=0.0,
                base=0,
                pattern=[[0, 2], [1, 128]],
                channel_multiplier=-1,
            )
            # ones column of V_aug
            nc.vector.memset(V_sb[:, :, :, D], 1.0)

            # ---- fill units: self-contained PE work parcels that can be
            # interleaved into the exp-paced attention stretches ----

            # V projection for one t-tile (8 MMs + bias add, ~1.7us PE)
            def v_unit(tt):
                def f():
                    ps = ps_pool.tile([128, 512], f32, tag="s")
                    q, off = tt // 4, (tt % 4) * 128
                    for cc in range(CCH):
                        nc.tensor.matmul(
                            ps[:],
                            lhsT=xT_sb[:, q, cc, off : off + 128],
                            rhs=wv_sb[:, cc, :],
                            start=(cc == 0),
                            stop=(cc == CCH - 1),
                        )
                    nc.vector.tensor_add(
                        V_sb[:, tt, :, 0:D],
                        ps[:].rearrange("p (h d) -> p h d", h=HL),
                        bv_sb[:].rearrange("p (h d) -> p h d", h=HL),
                    )
                return f

            # Q or K projection for one head pair / one 1024-wide t half
            # (16 MMs + bias add, ~3.4us PE)
            def qk_unit(pair, th, which):
                def f():
                    w_sb, dst, b_sb = (
                        (wq_sb, QT_sb, bq_sb),
                        (wk_sb, KT_sb, bk_sb),
                    )[which]
                    ps = ps_pool.tile([128, 2, 512], f32, tag="s")
                    for h2 in range(2):
                        q = th * 2 + h2
                        for cc in range(CCH):
                            nc.tensor.matmul(
                                ps[:, h2, :],
                                lhsT=w_sb[:, pair, cc, :],
                                rhs=xT_sb[:, q, cc, :],
                                start=(cc == 0),
                                stop=(cc == CCH - 1),
                            )
                    nc.vector.tensor_scalar_add(
                        dst[:, pair, th * 1024 : (th + 1) * 1024],
                        ps[:].rearrange("p a b -> p (a b)"),
                        b_sb[:, pair : pair + 1],
                    )
                return f

            # half a projection t-tile (4 MMs + copy, DMA on the second
            # half, ~0.9us PE)
            def proj_unit(tt, nh, so_box):
                def f():
                    if nh == 0:
                        so = st_pool.tile([128, 1024], bf, tag="so", name="so")
                        so_box["t"] = so
                    so = so_box["t"]
                    ps = ps_pool.tile([128, 512], f32, tag="s")
                    for cc in range(PCH):
                        nc.tensor.matmul(
                            ps[:],
                            lhsT=AT_sb[:, cc, tt * 128 : (tt + 1) * 128],
                            rhs=wp_sb[:, cc, nh * 512 : (nh + 1) * 512],
                            start=(cc == 0),
                            stop=(cc == PCH - 1),
                        )
                    nc.vector.tensor_copy(so[:, nh * 512 : (nh + 1) * 512], ps[:])
                    if nh == 1:
                        nc.sync.dma_start(
                            out=out[tt * 128 : (tt + 1) * 128, :], in_=so[:]
                        )
                return f

            # deadline-ordered fill queue; (ci, pair) lexicographic deadlines
            fills = []
            fseq = [0]

            def push_fill(deadline, fn):
                fills.append((deadline, fseq[0], fn))
                fseq[0] += 1
                fills.sort(key=lambda x: (x[0], x[1]))

            def proj_units(ci, deadline):
                for tt in range(4 * ci, 4 * ci + 4):
                    box = {}
                    for nh in range(2):
                        push_fill(deadline, proj_unit(tt, nh, box))

            def pop_fill():
                if fills:
                    fills.pop(0)[2]()

            def drain_fills(ci, pair):
                while fills and fills[0][0] <= (ci, pair):
                    fills.pop(0)[2]()

            # attention for one head pair / one 512-wide i chunk, with the
            # diagonal band trimmed to its live i-range
            def emit_att(pair, ci):
                o0 = o_ps.tile([D + 1, 512], f32, tag="o0")
                o1 = o_ps.tile([D + 1, 512], f32, tag="o1")
                njt = 4 * (ci + 1)

                # PV for tile jt (P read from SBUF pt, trimmed to [i0:512))
                def emit_pv(jt, i0, pt):
                    for s, ot in enumerate((o0, o1)):
                        nc.tensor.matmul(
                            ot[:, i0:512],
                            lhsT=V_sb[:, jt, 2 * pair + s, :],
                            rhs=pt[:, s, i0:512],
                            start=(jt == 0),
                            stop=(jt == njt - 1),
                            skip_group_check=True,
                        )

                # software-pipelined: the (always-ready) S pair for jt+1 is
                # emitted before PV for jt, so PV never blocks the PE queue
                # head while exp(jt) is still draining
                prev = None
                for jt in range(njt):
                    r = jt - 4 * ci
                    i0 = max(0, 128 * r)  # cols i < 128r are fully masked
                    st = ps_pool.tile([128, 2, 512], f32, tag="s")
                    for s in range(2):
                        nc.tensor.matmul(
                            st[:, s, i0:512],
                            lhsT=KT_sb[
                                64 * s : 64 * (s + 1),
                                pair,
                                jt * 128 : (jt + 1) * 128,
                            ],
                            rhs=QT_sb[
                                64 * s : 64 * (s + 1),
                                pair,
                                ci * 512 + i0 : (ci + 1) * 512,
                            ],
                            start=True,
                            stop=True,
                        )
                    pt = p_pool.tile([128, 2, 512], bf, tag="p")
                    nc.scalar.activation(
                        pt[:, :, i0:512], st[:, :, i0:512], Exp, scale=0.125
                    )
                    if r >= 0:
                        nc.vector.tensor_mul(
                            pt[:, :, i0 : i0 + 128],
                            pt[:, :, i0 : i0 + 128],
                            tri_sb[:],
                        )
                    if prev is not None:
                        emit_pv(*prev)
                    prev = (jt, i0, pt)
                    if jt % 3 == 1 and jt < 4 * ci:
                        pop_fill()
                emit_pv(*prev)
                # early PSUM->SBUF copies free the (single-buffered) o slots;
                # the PSUM reads legally shift head1's rows to partitions
                # 64-127 so the rest of the chain is partition-aligned
                oco = st_pool.tile([128, 512], f32, tag="oc")
                for s, ot in enumerate((o0, o1)):
                    nc.vector.tensor_copy(oco[64 * s : 64 * (s + 1), :], ot[0:D, :])
                    dn = r_pool.tile([1, 512], f32, tag=f"dn{s}")
                    nc.vector.tensor_copy(dn[:], ot[D : D + 1, :])
                    rc = r_pool.tile([1, 512], f32, tag=f"rc{s}")
                    nc.vector.reciprocal_approx_fast(rc[:], dn[:])
                    rb = r_pool.tile([128, 512], f32, tag=f"rb{s}")
                    nc.gpsimd.partition_broadcast(rb[:], rc[:])
                    nc.vector.tensor_mul(
                        AT_sb[
                            64 * s : 64 * (s + 1),
                            pair,
                            ci * 512 : (ci + 1) * 512,
                        ],
                        oco[64 * s : 64 * (s + 1), :],
                        rb[64 * s : 64 * (s + 1), :],
                    )

            # ---- main schedule ----
            # Structural (pre-attention) work: V tiles 0-3 and pair-0 th0 QK.
            # Everything else enters the fill queue and is popped one unit
            # per ~5 attention tiles, with deadline drains before the
            # attention that needs it.
            # Fill distribution balances each chunk's PE work against its exp
            # load (16/32/48/64 tiles): V tiles spread one per pair boundary
            # a chunk ahead of use, th1 QK lands in chunk 2, and the
            # (deadline-free) projections sort last in the queue so the
            # hooks pop them during exp-heavy chunk 3.
            for tt in range(4):
                v_unit(tt)()
            qk_unit(0, 0, 0)()
            qk_unit(0, 0, 1)()
            for p in range(1, NPAIR):
                for w in (0, 1):
                    push_fill((0, p), qk_unit(p, 0, w))
            vdl = [(0, 1), (0, 2), (0, 3), (1, 0)]
            for k, tt in enumerate(range(4, 8)):
                push_fill(vdl[k], v_unit(tt))
            for ci in range(NI):
                if ci == 1:
                    for k, tt in enumerate(range(8, 12)):
                        push_fill((1 + vdl[k][0], vdl[k][1]), v_unit(tt))
                    for p in range(NPAIR):
                        for w in (0, 1):
                            push_fill((2, p), qk_unit(p, 1, w))
                if ci == 2:
                    for k, tt in enumerate(range(12, 16)):
                        push_fill((2 + vdl[k][0], vdl[k][1]), v_unit(tt))
                for pair in range(NPAIR):
                    drain_fills(ci, pair)
                    emit_att(pair, ci)
                    if ci >= 1 and pair == 0:
                        # previous chunk's projection (AT fully written);
                        # no deadline -> consumed by late-chunk hooks
                        proj_units(ci - 1, (90 + ci, 0))
            drain_fills(99, 99)
            proj_units(NI - 1, (99, 100))
            drain_fills(99, 100)
    return nc


def _get_compiled():
    global _compiled
    if _compiled is None:
        from concourse import bacc

        nc = bacc.Bacc(
            "TRN2", target_bir_lowering=False, debug=False, num_devices=N_CORES
        )
        _build(nc)
        nc.compile()
        _compiled = nc
    return _compiled


def _shard_inputs(x, w_qkv, b_qkv, w_proj):
    """Build the 8 per-core input dicts (host-side transpose/slice/cast)."""
    in_maps = []
    wq_f, wk_f, wv_f = w_qkv[:, :C], w_qkv[:, C : 2 * C], w_qkv[:, 2 * C :]
    for c in range(N_CORES):
        b, g = c // 2, c % 2
        sl = slice(g * CL, (g + 1) * CL)
        bqs = np.ascontiguousarray(b_qkv[0 * C :][sl].reshape(NPAIR, 128).T)
        bks = np.ascontiguousarray(b_qkv[1 * C :][sl].reshape(NPAIR, 128).T)
        bvs = np.ascontiguousarray(
            np.broadcast_to(b_qkv[2 * C :][sl][None, :], (128, CL))
        )
        # per-pair packed qk weights: [NPAIR, 128 part, CCH*128], where the
        # partition index runs over the 128 rows of each 128-chunk of C
        wq_p = np.ascontiguousarray(
            wq_f[:, sl].reshape(CCH, 128, NPAIR, 128).transpose(2, 1, 0, 3)
            .reshape(NPAIR, 128, CCH * 128)
        )
        wk_p = np.ascontiguousarray(
            wk_f[:, sl].reshape(CCH, 128, NPAIR, 128).transpose(2, 1, 0, 3)
            .reshape(NPAIR, 128, CCH * 128)
        )
        # xT quarter-major: [128 part, 4 q, CCH, 512]
        xT_p = np.ascontiguousarray(
            x[b].T.reshape(CCH, 128, 4, 512).transpose(1, 2, 0, 3)
            .reshape(128, 4 * CCH * 512)
        )
        # wv: [128 part, CCH, CL]
        wv_p = np.ascontiguousarray(
            wv_f[:, sl].reshape(CCH, 128, CL).transpose(1, 0, 2)
            .reshape(128, CCH * CL)
        )
        in_maps.append(
            {
                "xT": xT_p.astype(BF16),
                "wq": wq_p.astype(BF16),
                "wk": wk_p.astype(BF16),
                "wv": wv_p.astype(BF16),
                "bq": bqs.astype(np.float32),
                "bk": bks.astype(np.float32),
                "bv": bvs.astype(np.float32),
                "wp": np.ascontiguousarray(w_proj[sl, :]).astype(BF16),
            }
        )
    return in_maps


def kernel(x, w_qkv, b_qkv, w_proj, b_proj, _trace=False, _tmpdir=None):
    from concourse.bass_utils import run_bass_kernel_spmd

    x = np.asarray(x, dtype=np.float32)
    w_qkv = np.asarray(w_qkv, dtype=np.float32)
    b_qkv = np.asarray(b_qkv, dtype=np.float32)
    w_proj = np.asarray(w_proj, dtype=np.float32)
    b_proj = np.asarray(b_proj, dtype=np.float32)

    nc = _get_compiled()
    in_maps = _shard_inputs(x, w_qkv, b_qkv, w_proj)
    res = run_bass_kernel_spmd(
        nc,
        in_maps,
        core_ids=list(range(N_CORES)),
        trace=_trace,
        tmpdir=_tmpdir,
    )
    out = np.empty((B, T, C), dtype=np.float32)
    for b in range(B):
        out[b] = (
            res.results[2 * b]["out"].astype(np.float32)
            + res.results[2 * b + 1]["out"].astype(np.float32)
            + b_proj
        )
    kernel._last_result = res
    return out
